# revision 14
# baseline (speedup 1.0000x reference)
"""Tensor-parallel llama-style attention (prefill) on 8 TRN2 NeuronCores.

bf16 version. All matmul operands are bf16: the PE runs bf16 at the same
1 cycle/row as f32r, but LDWEIGHTS halves (256 vs 512+ cycles per
[128,128] tile) so stationary loads hide completely under the 512-cycle
moving streams, and DMA/SBUF traffic halves. fp8 was evaluated and
rejected: e4m3's 3.6% per-element quantization error is multiplicative
into the output (sim: 5-9e-2 rel vs the 2e-2 budget; bf16 sims at 4e-3).

Sharding: tensor-parallel over heads. Core c holds q-heads [4c, 4c+4),
kv-head c, the matching rows of wq/wk/wv, and columns [512c, 512c+512) of
wo. Each core computes a full-size partial of the output projection;
partials are summed on the host (the "all-reduce after wo").

Device-side layout:
  - Activations keep the feature dim on partitions: xT [DIM, TOK],
    Q^T/K^T [128, S] per head, V in token-major chunks. Q is SBUF-
    resident in bf16 (no DRAM spill). wo is loaded once and stays
    resident.
  - RoPE: head-dim basis permuted on the host (even components first,
    odd second) -> half-partition swap + mul/add vs cos/sin tables,
    reading the projection result straight from PSUM.
  - Causal masking: the 4 distinct diagonal mask tiles (relative key
    offset 0/128/256/384 vs a 512-query block) are built once on-chip
    with gpsimd.affine_select and added on DVE - no mask DMA at all.
    Non-causal masks fall back to a DVE tensor_add of a DMA'd mask.
  - Softmax: no max-subtraction (scores*scale is O(10); exp in f32 is
    safe). Row sums via a [128,128] ones matmul on the PE, which
    replicates the denominator across all 128 PSUM partitions for free
    (PE cycles scale with moving rows, not output width), so no
    cross-partition broadcast is needed; 1/sum via the 5x-faster
    reciprocal_approx_fast (~18 correct bits, plenty for 2e-2).
  - Output projection: token-tile outer / column-block inner over all 8
    PSUM banks, so each att stationary tile serves 8 matmuls, and each
    token tile leaves as one contiguous [128, 4096] bf16 DMA.
"""

import math
import os
import sys

sys.path.insert(0, "/opt/trn_rl_repo")

import numpy as np
import ml_dtypes

import concourse.bacc as bacc
import concourse.tile as tile
import concourse.mybir as mybir
from concourse import masks
from concourse.bass_utils import run_bass_kernel_spmd

B, S, DIM = 2, 2048, 4096
TOK = B * S
NH, NKV, HD = 32, 8, 128
NCORES = 8
HQ = NH // NCORES            # 4 query heads per core
SCALE = 1.0 / math.sqrt(HD)
F32 = mybir.dt.float32
BF16 = mybir.dt.bfloat16
NP_BF = ml_dtypes.bfloat16
EXP = mybir.ActivationFunctionType.Exp
GE = mybir.AluOpType.is_ge

QB = 4          # q-blocks per batch (512 queries each)
QW = S // QB    # 512
KT = S // 128   # 16 k-tiles per batch
NJ = HQ + 2     # 6 projection output tiles: 4 Q heads, K, V


def _build(causal: bool):
    nc = bacc.Bacc("TRN2", target_bir_lowering=False, debug=False)

    xT_d = nc.dram_tensor("xT", [DIM, TOK], BF16, kind="ExternalInput")
    w_d = nc.dram_tensor("wqkvT", [DIM, NJ * HD], BF16, kind="ExternalInput")
    wo_d = nc.dram_tensor("woT", [HQ * HD, DIM], BF16, kind="ExternalInput")
    cos_d = nc.dram_tensor("cosT", [HD, S], F32, kind="ExternalInput")
    sin_d = nc.dram_tensor("sinTs", [HD, S], F32, kind="ExternalInput")
    if not causal:
        mask_d = nc.dram_tensor("maskTd", [QB * KT, 128, QW], F32,
                                kind="ExternalInput")
    out_d = nc.dram_tensor("out_part", [TOK, DIM], BF16,
                           kind="ExternalOutput")

    xT = xT_d.ap().rearrange("(kt p) t -> p kt t", p=128)      # [128, 32, TOK]
    w_ap = w_d.ap().rearrange("(kt p) j -> p kt j", p=128)     # [128, 32, 768]
    wo_ap = wo_d.ap().rearrange("(dt p) m -> p dt m", p=128)   # [128, 4, DIM]
    out_v = out_d.ap().rearrange("(g p) m -> p g m", p=128)    # [128, 32, DIM]

    with tile.TileContext(nc) as tc:
        with (
            tc.tile_pool(name="const", bufs=1) as const_pool,
            tc.tile_pool(name="batch", bufs=1) as batch_pool,
            tc.tile_pool(name="kv", bufs=2) as kv_pool,
        ):
            wqkv = const_pool.tile([128, 32, NJ * HD], BF16)
            for kc in range(4):     # chunked so P1 can start early
                nc.scalar.dma_start(wqkv[:, kc * 8:(kc + 1) * 8, :],
                                    w_ap[:, kc * 8:(kc + 1) * 8, :])
            wo_s = const_pool.tile([128, HQ, DIM], BF16)
            for mc in range(4):
                nc.scalar.dma_start(
                    wo_s[:, :, mc * 1024:(mc + 1) * 1024],
                    wo_ap[:, :, mc * 1024:(mc + 1) * 1024])
            ident = const_pool.tile([128, 128], F32)
            masks.make_identity(nc, ident[:])
            ones_f = const_pool.tile([128, 128], F32)
            nc.vector.memset(ones_f[:], 1.0)
            # [128,128] ones stationary: the sums matmul then replicates
            # the softmax denominator across all 128 PSUM partitions at no
            # extra PE cost (cycles scale with moving rows, not out width),
            # killing the gpsimd partition_broadcast.
            ones_sq = const_pool.tile([128, 128], BF16)
            nc.vector.tensor_copy(ones_sq[:], ones_f[:])
            if causal:
                # diagonal mask tiles built on-chip: tile j covers keys
                # [128j, 128j+128) vs queries [0, 512) relative to the
                # block diagonal; keep 0 where q >= p + 128j, else -1e9
                m4 = const_pool.tile([128, 4, QW], F32)
                nc.gpsimd.memset(m4[:], 0.0)
                for j in range(4):
                    nc.gpsimd.affine_select(
                        out=m4[:, j, :], in_=m4[:, j, :],
                        compare_op=GE, fill=-1e9,
                        base=-128 * j, pattern=[[1, QW]],
                        channel_multiplier=-1)

            # per-batch SBUF-resident activations; K/V double-buffered so
            # P1(b+1) can overwrite them while A(b) is still reading
            qh_s = batch_pool.tile([128, HQ, S], BF16)
            att_h = batch_pool.tile([128, HQ, S], BF16)

            for b in range(B):
                kT_s = kv_pool.tile([128, S], BF16, tag="kT")
                v_s = kv_pool.tile([128, KT, HD], BF16, tag="v")
                # ---------- P1: QKV projections + RoPE ----------
                with (
                    tc.tile_pool(name="xt", bufs=2) as xt_pool,
                    tc.tile_pool(name="cs", bufs=2) as cs_pool,
                    tc.tile_pool(name="rope", bufs=2) as rope_pool,
                    tc.tile_pool(name="vtmp", bufs=2) as vtmp_pool,
                    tc.tile_pool(name="p1ps", bufs=NJ, space="PSUM") as p1ps,
                    tc.tile_pool(name="trps", bufs=2, space="PSUM") as trps,
                ):
                    for tb in range(4):          # 512-token chunks
                        c0 = b * S + tb * 512
                        sl = slice(tb * 512, tb * 512 + 512)
                        cos_c = cs_pool.tile([HD, 512], F32, tag="cos")
                        sin_c = cs_pool.tile([HD, 512], F32, tag="sin")
                        nc.sync.dma_start(cos_c[:], cos_d.ap()[:, sl])
                        nc.sync.dma_start(sin_c[:], sin_d.ap()[:, sl])
                        pss = [p1ps.tile([128, 512], F32, tag="ps",
                                         name=f"ps{j}")
                               for j in range(NJ)]
                        for ks in range(4):      # k slices of 8 x-tiles
                            xt_c = xt_pool.tile([128, 8, 512], BF16, tag="xt")
                            nc.sync.dma_start(
                                xt_c[:],
                                xT[:, ks * 8:(ks + 1) * 8, c0:c0 + 512])
                            for j in range(NJ):
                                for k in range(8):
                                    nc.tensor.matmul(
                                        pss[j][:],
                                        wqkv[:, ks * 8 + k,
                                             j * HD:(j + 1) * HD],
                                        xt_c[:, k, :],
                                        start=(ks == 0 and k == 0),
                                        stop=(ks == 3 and k == 7))
                        for j in range(NJ):
                            ps = pss[j]
                            if j < HQ + 1:
                                # RoPE: out = z*cos + swap64(z)*sin_signed
                                tmp = rope_pool.tile([128, 512], F32,
                                                     tag="tmp")
                                nc.vector.tensor_mul(
                                    tmp[0:64, :], ps[64:128, :],
                                    sin_c[0:64, :])
                                nc.vector.tensor_mul(
                                    tmp[64:128, :], ps[0:64, :],
                                    sin_c[64:128, :])
                                t2 = rope_pool.tile([128, 512], F32, tag="t2")
                                nc.vector.tensor_mul(t2[:], ps[:], cos_c[:])
                                if j < HQ:
                                    nc.vector.tensor_add(
                                        qh_s[:, j, sl], t2[:], tmp[:])
                                else:
                                    nc.vector.tensor_add(
                                        kT_s[:, sl], t2[:], tmp[:])
                            else:
                                # V: copy from PSUM, transpose to token-major
                                v_sb = vtmp_pool.tile([128, 512], F32)
                                nc.vector.tensor_copy(v_sb[:], ps[:])
                                for h2 in range(4):
                                    tp = trps.tile([128, 128], F32)
                                    nc.tensor.transpose(
                                        tp[:],
                                        v_sb[:, h2 * 128:(h2 + 1) * 128],
                                        ident[:])
                                    nc.vector.tensor_copy(
                                        v_s[:, tb * 4 + h2, :], tp[:])

                # ---------- A: attention (writes att_h in SBUF) ----------
                with (
                    tc.tile_pool(name="mask", bufs=2) as mask_pool,
                    tc.tile_pool(name="pT", bufs=4) as p_pool,
                    tc.tile_pool(name="rcp", bufs=2) as r_pool,
                    tc.tile_pool(name="sps", bufs=3, space="PSUM") as sps,
                    tc.tile_pool(name="sums", bufs=2, space="PSUM") as sums_ps,
                    tc.tile_pool(name="ops", bufs=3, space="PSUM") as o_ps_pool,
                ):
                    for qb in range(QB):
                        if causal:
                            nkt = 4 * (qb + 1)
                            kt0 = 4 * qb
                        else:
                            m_s = mask_pool.tile([128, KT, QW], F32)
                            nc.scalar.dma_start(
                                m_s[:],
                                mask_d.ap()[qb * KT:(qb + 1) * KT]
                                .rearrange("kt p q -> p kt q"))
                            nkt = KT
                            kt0 = 0
                        for h in range(HQ):
                            qh = qh_s[:, h, qb * QW:(qb + 1) * QW]
                            sum_ps = sums_ps.tile([128, QW], F32)
                            o_ps = o_ps_pool.tile([128, QW], F32)
                            # software-pipelined by one kt: the PE issues
                            # scores(kt+1) while ACT exps pT(kt)
                            prev = None
                            for kt in range(nkt):
                                s_ps = sps.tile([128, QW], F32, tag="s_ps")
                                nc.tensor.matmul(
                                    s_ps[:], kT_s[:, kt * 128:(kt + 1) * 128],
                                    qh, start=True, stop=True)
                                if causal:
                                    if kt >= kt0:
                                        nc.vector.tensor_add(
                                            s_ps[:], s_ps[:],
                                            m4[:, kt - kt0, :])
                                else:
                                    nc.vector.tensor_add(
                                        s_ps[:], s_ps[:], m_s[:, kt, :])
                                pT = p_pool.tile([128, QW], BF16, tag="pT")
                                nc.scalar.activation(
                                    pT[:], s_ps[:], EXP, bias=0.0,
                                    scale=SCALE)
                                if prev is not None:
                                    pv, pkt = prev
                                    nc.tensor.matmul(
                                        sum_ps[:], ones_sq[:], pv[:],
                                        start=(pkt == 0), stop=False)
                                    nc.tensor.matmul(
                                        o_ps[:], v_s[:, pkt, :], pv[:],
                                        start=(pkt == 0), stop=False)
                                prev = (pT, kt)
                            pv, pkt = prev
                            nc.tensor.matmul(
                                sum_ps[:], ones_sq[:], pv[:],
                                start=(pkt == 0), stop=True)
                            nc.tensor.matmul(
                                o_ps[:], v_s[:, pkt, :], pv[:],
                                start=(pkt == 0), stop=True)
                            bc_sb = r_pool.tile([128, QW], F32, tag="bc")
                            nc.vector.reciprocal_approx_fast(
                                bc_sb[:], sum_ps[:])
                            nc.vector.tensor_mul(
                                att_h[:, h, qb * QW:(qb + 1) * QW],
                                o_ps[:], bc_sb[:])

                # ---------- W: output projection partial ----------
                # token-tile outer, mb inner: each att stationary tile is
                # reused by 8 consecutive matmuls (one per 512-wide output
                # column block) across all 8 PSUM banks, and each token
                # tile drains as one contiguous [128, 4096] DMA.
                with (
                    tc.tile_pool(name="osb", bufs=2) as osb_pool,
                    tc.tile_pool(name="wps", bufs=8, space="PSUM") as wps,
                ):
                    for tt in range(16):         # 128-token tiles
                        ps_ws = [wps.tile([128, 512], F32, tag="w",
                                          name=f"w{mb}")
                                 for mb in range(8)]
                        for d4 in range(HQ):
                            for mb in range(8):
                                nc.tensor.matmul(
                                    ps_ws[mb][:],
                                    att_h[:, d4, tt * 128:(tt + 1) * 128],
                                    wo_s[:, d4, mb * 512:(mb + 1) * 512],
                                    start=(d4 == 0), stop=(d4 == HQ - 1))
                        o_sb = osb_pool.tile([128, 8, 512], BF16)
                        for mb in range(8):
                            # split PSUM->SBUF casts between DVE and ACT
                            if mb % 2 == 0:
                                nc.vector.tensor_copy(
                                    o_sb[:, mb, :], ps_ws[mb][:])
                            else:
                                nc.scalar.activation(
                                    o_sb[:, mb, :], ps_ws[mb][:],
                                    mybir.ActivationFunctionType.Copy)
                        g = b * (S // 128) + tt
                        nc.sync.dma_start(
                            out_v[:, g:g + 1, :],
                            o_sb[:].rearrange("p mb q -> p (mb q)"))

    nc.compile()
    return nc


_CACHE = {}
LAST_EXEC_NS = None


def _get_nc(causal: bool):
    if causal not in _CACHE:
        _CACHE[causal] = _build(causal)
    return _CACHE[causal]


def _host_prep(x, wq, wk, wv, wo, freqs_cos, freqs_sin, mask):
    perm = np.concatenate([np.arange(0, HD, 2), np.arange(1, HD, 2)])
    wq_p = wq.reshape(NH, HD, DIM)[:, perm, :].reshape(NH * HD, DIM)
    wk_p = wk.reshape(NKV, HD, DIM)[:, perm, :].reshape(NKV * HD, DIM)

    xT = np.ascontiguousarray(x.reshape(TOK, DIM).T).astype(NP_BF)

    cos = freqs_cos.T                     # [64, S]
    sin = freqs_sin.T
    cosT = np.ascontiguousarray(np.concatenate([cos, cos], 0))       # [128, S]
    sinTs = np.ascontiguousarray(np.concatenate([-sin, sin], 0))

    ref_mask = np.triu(np.full((S, S), -1e9, dtype=np.float32), k=1)
    causal = np.array_equal(mask, ref_mask)

    if not causal:
        maskT = np.ascontiguousarray(mask.T) / np.float32(SCALE)   # [k, q]
        maskTd = np.empty((QB * KT, 128, QW), dtype=np.float32)
        for qb in range(QB):
            for j in range(KT):
                maskTd[qb * KT + j] = maskT[j * 128:(j + 1) * 128,
                                            qb * QW:(qb + 1) * QW]

    in_maps = []
    for c in range(NCORES):
        wqT = wq_p[c * HQ * HD:(c + 1) * HQ * HD, :].T          # [DIM, 512]
        wkT = wk_p[c * HD:(c + 1) * HD, :].T                    # [DIM, 128]
        wvT = wv[c * HD:(c + 1) * HD, :].T                      # [DIM, 128]
        wqkvT = np.ascontiguousarray(
            np.concatenate([wqT, wkT, wvT], 1)).astype(NP_BF)
        woT = np.ascontiguousarray(
            wo[:, c * HQ * HD:(c + 1) * HQ * HD].T).astype(NP_BF)
        m = {"xT": xT, "wqkvT": wqkvT, "woT": woT,
             "cosT": cosT, "sinTs": sinTs}
        if not causal:
            m["maskTd"] = maskTd
        in_maps.append(m)
    return causal, in_maps


def kernel(x, wq, wk, wv, wo, freqs_cos, freqs_sin, mask, start_pos):
    global LAST_EXEC_NS
    causal, in_maps = _host_prep(
        np.asarray(x, np.float32), np.asarray(wq, np.float32),
        np.asarray(wk, np.float32), np.asarray(wv, np.float32),
        np.asarray(wo, np.float32), np.asarray(freqs_cos, np.float32),
        np.asarray(freqs_sin, np.float32), np.asarray(mask, np.float32))

    nc = _get_nc(causal)
    res = run_bass_kernel_spmd(nc, in_maps, core_ids=list(range(NCORES)))
    LAST_EXEC_NS = res.exec_time_ns

    acc = res.results[0]["out_part"].astype(np.float64)
    for c in range(1, NCORES):
        acc += res.results[c]["out_part"].astype(np.float64)
    return acc.astype(np.float32).reshape(B, S, DIM)


if __name__ == "__main__":
    rng = np.random.default_rng(0)
    inputs = {
        "x": rng.standard_normal((B, S, DIM), dtype=np.float32),
        "wq": (rng.standard_normal((DIM, DIM), dtype=np.float32) * 0.02),
        "wk": (rng.standard_normal((NKV * HD, DIM), dtype=np.float32) * 0.02),
        "wv": (rng.standard_normal((NKV * HD, DIM), dtype=np.float32) * 0.02),
        "wo": (rng.standard_normal((DIM, DIM), dtype=np.float32) * 0.02),
        "freqs_cos": rng.random((S, HD // 2), dtype=np.float32),
        "freqs_sin": rng.random((S, HD // 2), dtype=np.float32),
        "mask": np.triu(np.full((S, S), -1e9, dtype=np.float32), k=1),
        "start_pos": 0,
    }
    out = kernel(**inputs)
    print("out", out.shape, out.dtype, float(np.abs(out).mean()))


# revision 16
# speedup vs baseline: 1.1627x; 1.1627x over previous
"""Tensor-parallel llama-style attention (prefill) on 8 TRN2 NeuronCores.

bf16 version. All matmul operands are bf16: the PE runs bf16 at the same
1 cycle/row as f32r, but LDWEIGHTS halves (256 vs 512+ cycles per
[128,128] tile) so stationary loads hide completely under the 512-cycle
moving streams, and DMA/SBUF traffic halves. fp8 was evaluated and
rejected: e4m3's 3.6% per-element quantization error is multiplicative
into the output (sim: 5-9e-2 rel vs the 2e-2 budget; bf16 sims at 4e-3).

Sharding: tensor-parallel over heads. Core c holds q-heads [4c, 4c+4),
kv-head c, the matching rows of wq/wk/wv, and columns [512c, 512c+512) of
wo. Each core computes a full-size partial of the output projection;
partials are summed on the host (the "all-reduce after wo").

Device-side layout:
  - Activations keep the feature dim on partitions: xT [DIM, TOK],
    Q^T/K^T [128, S] per head, V in token-major chunks. Q is SBUF-
    resident in bf16 (no DRAM spill). wo is loaded once and stays
    resident.
  - RoPE: head-dim basis permuted on the host (even components first,
    odd second) -> half-partition swap + mul/add vs cos/sin tables,
    reading the projection result straight from PSUM.
  - Causal masking: the 4 distinct diagonal mask tiles (relative key
    offset 0/128/256/384 vs a 512-query block) are built once on-chip
    with gpsimd.affine_select and added on DVE - no mask DMA at all.
    Diagonal score tiles are also narrowed to the visible query range
    [128j, 512), cutting PE/ACT/DVE work on the diagonal by ~38% with
    no extra instructions; the skipped region is never read.
    Non-causal masks fall back to a DVE tensor_add of a DMA'd mask.
  - Softmax: no max-subtraction (scores*scale is O(10); exp in f32 is
    safe). Row sums via a ones-column matmul on the PE (DVE cannot
    reduce across partitions); 1/sum via reciprocal_approx_fast (5x
    faster than reciprocal, ~18 correct bits - plenty for 2e-2), then
    gpsimd.partition_broadcast.
"""

import math
import os
import sys

sys.path.insert(0, "/opt/trn_rl_repo")

import numpy as np
import ml_dtypes

import concourse.bacc as bacc
import concourse.tile as tile
import concourse.mybir as mybir
from concourse import masks
from concourse.bass_utils import run_bass_kernel_spmd

B, S, DIM = 2, 2048, 4096
TOK = B * S
NH, NKV, HD = 32, 8, 128
NCORES = 8
HQ = NH // NCORES            # 4 query heads per core
SCALE = 1.0 / math.sqrt(HD)
F32 = mybir.dt.float32
BF16 = mybir.dt.bfloat16
NP_BF = ml_dtypes.bfloat16
EXP = mybir.ActivationFunctionType.Exp
GE = mybir.AluOpType.is_ge

QB = 4          # q-blocks per batch (512 queries each)
QW = S // QB    # 512
KT = S // 128   # 16 k-tiles per batch
NJ = HQ + 2     # 6 projection output tiles: 4 Q heads, K, V


def _build(causal: bool):
    nc = bacc.Bacc("TRN2", target_bir_lowering=False, debug=False)

    xT_d = nc.dram_tensor("xT", [DIM, TOK], BF16, kind="ExternalInput")
    w_d = nc.dram_tensor("wqkvT", [DIM, NJ * HD], BF16, kind="ExternalInput")
    wo_d = nc.dram_tensor("woT", [HQ * HD, DIM], BF16, kind="ExternalInput")
    cos_d = nc.dram_tensor("cosT", [HD, S], F32, kind="ExternalInput")
    sin_d = nc.dram_tensor("sinTs", [HD, S], F32, kind="ExternalInput")
    if not causal:
        mask_d = nc.dram_tensor("maskTd", [QB * KT, 128, QW], F32,
                                kind="ExternalInput")
    out_d = nc.dram_tensor("out_part", [TOK, DIM], BF16,
                           kind="ExternalOutput")

    xT = xT_d.ap().rearrange("(kt p) t -> p kt t", p=128)      # [128, 32, TOK]
    w_ap = w_d.ap().rearrange("(kt p) j -> p kt j", p=128)     # [128, 32, 768]
    wo_ap = wo_d.ap().rearrange("(dt p) m -> p dt m", p=128)   # [128, 4, DIM]
    out_v = out_d.ap().rearrange("(g p) m -> p g m", p=128)    # [128, 32, DIM]

    with tile.TileContext(nc) as tc:
        with (
            tc.tile_pool(name="const", bufs=1) as const_pool,
            tc.tile_pool(name="batch", bufs=1) as batch_pool,
            tc.tile_pool(name="kv", bufs=2) as kv_pool,
        ):
            wqkv = const_pool.tile([128, 32, NJ * HD], BF16)
            for kc in range(4):     # chunked so P1 can start early
                nc.scalar.dma_start(wqkv[:, kc * 8:(kc + 1) * 8, :],
                                    w_ap[:, kc * 8:(kc + 1) * 8, :])
            wo_s = const_pool.tile([128, HQ, DIM], BF16)
            for mc in range(4):
                nc.scalar.dma_start(
                    wo_s[:, :, mc * 1024:(mc + 1) * 1024],
                    wo_ap[:, :, mc * 1024:(mc + 1) * 1024])
            ident = const_pool.tile([128, 128], F32)
            masks.make_identity(nc, ident[:])
            ones_f = const_pool.tile([128, 1], F32)
            nc.vector.memset(ones_f[:], 1.0)
            ones_col = const_pool.tile([128, 1], BF16)
            nc.vector.tensor_copy(ones_col[:], ones_f[:])
            if causal:
                # diagonal mask tiles built on-chip: tile j covers keys
                # [128j, 128j+128) vs queries [0, 512) relative to the
                # block diagonal; keep 0 where q >= p + 128j, else -1e9
                m4 = const_pool.tile([128, 4, QW], F32)
                nc.gpsimd.memset(m4[:], 0.0)
                for j in range(4):
                    nc.gpsimd.affine_select(
                        out=m4[:, j, :], in_=m4[:, j, :],
                        compare_op=GE, fill=-1e9,
                        base=-128 * j, pattern=[[1, QW]],
                        channel_multiplier=-1)

            # per-batch SBUF-resident activations; K/V double-buffered so
            # P1(b+1) can overwrite them while A(b) is still reading
            qh_s = batch_pool.tile([128, HQ, S], BF16)
            att_h = batch_pool.tile([128, HQ, S], BF16)

            for b in range(B):
                kT_s = kv_pool.tile([128, S], BF16, tag="kT")
                v_s = kv_pool.tile([128, KT, HD], BF16, tag="v")
                # ---------- P1: QKV projections + RoPE ----------
                with (
                    tc.tile_pool(name="xt", bufs=2) as xt_pool,
                    tc.tile_pool(name="cs", bufs=2) as cs_pool,
                    tc.tile_pool(name="rope", bufs=2) as rope_pool,
                    tc.tile_pool(name="vtmp", bufs=2) as vtmp_pool,
                    tc.tile_pool(name="p1ps", bufs=NJ, space="PSUM") as p1ps,
                    tc.tile_pool(name="trps", bufs=2, space="PSUM") as trps,
                ):
                    for tb in range(4):          # 512-token chunks
                        c0 = b * S + tb * 512
                        sl = slice(tb * 512, tb * 512 + 512)
                        cos_c = cs_pool.tile([HD, 512], F32, tag="cos")
                        sin_c = cs_pool.tile([HD, 512], F32, tag="sin")
                        nc.sync.dma_start(cos_c[:], cos_d.ap()[:, sl])
                        nc.sync.dma_start(sin_c[:], sin_d.ap()[:, sl])
                        pss = [p1ps.tile([128, 512], F32, tag="ps",
                                         name=f"ps{j}")
                               for j in range(NJ)]
                        for ks in range(4):      # k slices of 8 x-tiles
                            xt_c = xt_pool.tile([128, 8, 512], BF16, tag="xt")
                            nc.sync.dma_start(
                                xt_c[:],
                                xT[:, ks * 8:(ks + 1) * 8, c0:c0 + 512])
                            for j in range(NJ):
                                for k in range(8):
                                    nc.tensor.matmul(
                                        pss[j][:],
                                        wqkv[:, ks * 8 + k,
                                             j * HD:(j + 1) * HD],
                                        xt_c[:, k, :],
                                        start=(ks == 0 and k == 0),
                                        stop=(ks == 3 and k == 7))
                        for j in range(NJ):
                            ps = pss[j]
                            if j < HQ + 1:
                                # RoPE: out = z*cos + swap64(z)*sin_signed
                                tmp = rope_pool.tile([128, 512], F32,
                                                     tag="tmp")
                                nc.vector.tensor_mul(
                                    tmp[0:64, :], ps[64:128, :],
                                    sin_c[0:64, :])
                                nc.vector.tensor_mul(
                                    tmp[64:128, :], ps[0:64, :],
                                    sin_c[64:128, :])
                                t2 = rope_pool.tile([128, 512], F32, tag="t2")
                                nc.vector.tensor_mul(t2[:], ps[:], cos_c[:])
                                if j < HQ:
                                    nc.vector.tensor_add(
                                        qh_s[:, j, sl], t2[:], tmp[:])
                                else:
                                    nc.vector.tensor_add(
                                        kT_s[:, sl], t2[:], tmp[:])
                            else:
                                # V: copy from PSUM, transpose to token-major
                                v_sb = vtmp_pool.tile([128, 512], F32)
                                nc.vector.tensor_copy(v_sb[:], ps[:])
                                for h2 in range(4):
                                    tp = trps.tile([128, 128], F32)
                                    nc.tensor.transpose(
                                        tp[:],
                                        v_sb[:, h2 * 128:(h2 + 1) * 128],
                                        ident[:])
                                    nc.vector.tensor_copy(
                                        v_s[:, tb * 4 + h2, :], tp[:])

                # ---------- A: attention (writes att_h in SBUF) ----------
                with (
                    tc.tile_pool(name="mask", bufs=2) as mask_pool,
                    tc.tile_pool(name="pT", bufs=4) as p_pool,
                    tc.tile_pool(name="rcp", bufs=2) as r_pool,
                    tc.tile_pool(name="sps", bufs=3, space="PSUM") as sps,
                    tc.tile_pool(name="sums", bufs=2, space="PSUM") as sums_ps,
                    tc.tile_pool(name="ops", bufs=3, space="PSUM") as o_ps_pool,
                ):
                    for qb in range(QB):
                        if causal:
                            nkt = 4 * (qb + 1)
                            kt0 = 4 * qb
                        else:
                            m_s = mask_pool.tile([128, KT, QW], F32)
                            nc.scalar.dma_start(
                                m_s[:],
                                mask_d.ap()[qb * KT:(qb + 1) * KT]
                                .rearrange("kt p q -> p kt q"))
                            nkt = KT
                            kt0 = 0
                        for h in range(HQ):
                            sum_ps = sums_ps.tile([1, QW], F32)
                            o_ps = o_ps_pool.tile([128, QW], F32)
                            # software-pipelined by one kt: the PE issues
                            # scores(kt+1) while ACT exps pT(kt). Diagonal
                            # tiles are narrowed to the visible query range
                            # [128j, 512): the skipped region is never read.
                            prev = None
                            for kt in range(nkt):
                                w0 = (128 * (kt - kt0)
                                      if causal and kt >= kt0 else 0)
                                s_ps = sps.tile([128, QW], F32, tag="s_ps")
                                nc.tensor.matmul(
                                    s_ps[:, w0:],
                                    kT_s[:, kt * 128:(kt + 1) * 128],
                                    qh_s[:, h, qb * QW + w0:(qb + 1) * QW],
                                    start=True, stop=True)
                                if causal:
                                    if kt >= kt0:
                                        nc.vector.tensor_add(
                                            s_ps[:, w0:], s_ps[:, w0:],
                                            m4[:, kt - kt0, w0:])
                                else:
                                    nc.vector.tensor_add(
                                        s_ps[:], s_ps[:], m_s[:, kt, :])
                                pT = p_pool.tile([128, QW], BF16, tag="pT")
                                nc.scalar.activation(
                                    pT[:, w0:], s_ps[:, w0:], EXP, bias=0.0,
                                    scale=SCALE)
                                if prev is not None:
                                    pv, pkt, pw = prev
                                    nc.tensor.matmul(
                                        sum_ps[:, pw:], ones_col[:],
                                        pv[:, pw:],
                                        start=(pkt == 0), stop=False)
                                    nc.tensor.matmul(
                                        o_ps[:, pw:], v_s[:, pkt, :],
                                        pv[:, pw:],
                                        start=(pkt == 0), stop=False)
                                prev = (pT, kt, w0)
                            pv, pkt, pw = prev
                            nc.tensor.matmul(
                                sum_ps[:, pw:], ones_col[:], pv[:, pw:],
                                start=(pkt == 0), stop=True)
                            nc.tensor.matmul(
                                o_ps[:, pw:], v_s[:, pkt, :], pv[:, pw:],
                                start=(pkt == 0), stop=True)
                            rcp = r_pool.tile([1, QW], F32, tag="rcp")
                            nc.vector.reciprocal_approx_fast(
                                rcp[:], sum_ps[:])
                            bc_sb = r_pool.tile([128, QW], F32, tag="bc")
                            nc.gpsimd.partition_broadcast(bc_sb[:], rcp[:])
                            nc.vector.tensor_mul(
                                att_h[:, h, qb * QW:(qb + 1) * QW],
                                o_ps[:], bc_sb[:])

                # ---------- W: output projection partial ----------
                with (
                    tc.tile_pool(name="osb", bufs=2) as osb_pool,
                    tc.tile_pool(name="wps", bufs=5, space="PSUM") as wps,
                ):
                    for mb in range(8):          # 512-wide output columns
                        for tg in range(4):      # groups of 4 token tiles
                            o_sb = osb_pool.tile([128, 4, 512], BF16)
                            for ts in range(4):
                                tt = tg * 4 + ts
                                ps_w = wps.tile([128, 512], F32)
                                for d4 in range(HQ):
                                    nc.tensor.matmul(
                                        ps_w[:],
                                        att_h[:, d4, tt * 128:(tt + 1) * 128],
                                        wo_s[:, d4,
                                             mb * 512:(mb + 1) * 512],
                                        start=(d4 == 0), stop=(d4 == HQ - 1))
                                # split PSUM->SBUF casts between DVE and ACT
                                if ts % 2 == 0:
                                    nc.vector.tensor_copy(
                                        o_sb[:, ts, :], ps_w[:])
                                else:
                                    nc.scalar.activation(
                                        o_sb[:, ts, :], ps_w[:],
                                        mybir.ActivationFunctionType.Copy)
                            g0 = b * (S // 128) + tg * 4
                            nc.sync.dma_start(
                                out_v[:, g0:g0 + 4, mb * 512:(mb + 1) * 512],
                                o_sb[:])

    nc.compile()
    return nc


_CACHE = {}
LAST_EXEC_NS = None


def _get_nc(causal: bool):
    if causal not in _CACHE:
        _CACHE[causal] = _build(causal)
    return _CACHE[causal]


def _host_prep(x, wq, wk, wv, wo, freqs_cos, freqs_sin, mask):
    perm = np.concatenate([np.arange(0, HD, 2), np.arange(1, HD, 2)])
    wq_p = wq.reshape(NH, HD, DIM)[:, perm, :].reshape(NH * HD, DIM)
    wk_p = wk.reshape(NKV, HD, DIM)[:, perm, :].reshape(NKV * HD, DIM)

    xT = np.ascontiguousarray(x.reshape(TOK, DIM).T).astype(NP_BF)

    cos = freqs_cos.T                     # [64, S]
    sin = freqs_sin.T
    cosT = np.ascontiguousarray(np.concatenate([cos, cos], 0))       # [128, S]
    sinTs = np.ascontiguousarray(np.concatenate([-sin, sin], 0))

    ref_mask = np.triu(np.full((S, S), -1e9, dtype=np.float32), k=1)
    causal = np.array_equal(mask, ref_mask)

    if not causal:
        maskT = np.ascontiguousarray(mask.T) / np.float32(SCALE)   # [k, q]
        maskTd = np.empty((QB * KT, 128, QW), dtype=np.float32)
        for qb in range(QB):
            for j in range(KT):
                maskTd[qb * KT + j] = maskT[j * 128:(j + 1) * 128,
                                            qb * QW:(qb + 1) * QW]

    in_maps = []
    for c in range(NCORES):
        wqT = wq_p[c * HQ * HD:(c + 1) * HQ * HD, :].T          # [DIM, 512]
        wkT = wk_p[c * HD:(c + 1) * HD, :].T                    # [DIM, 128]
        wvT = wv[c * HD:(c + 1) * HD, :].T                      # [DIM, 128]
        wqkvT = np.ascontiguousarray(
            np.concatenate([wqT, wkT, wvT], 1)).astype(NP_BF)
        woT = np.ascontiguousarray(
            wo[:, c * HQ * HD:(c + 1) * HQ * HD].T).astype(NP_BF)
        m = {"xT": xT, "wqkvT": wqkvT, "woT": woT,
             "cosT": cosT, "sinTs": sinTs}
        if not causal:
            m["maskTd"] = maskTd
        in_maps.append(m)
    return causal, in_maps


def kernel(x, wq, wk, wv, wo, freqs_cos, freqs_sin, mask, start_pos):
    global LAST_EXEC_NS
    causal, in_maps = _host_prep(
        np.asarray(x, np.float32), np.asarray(wq, np.float32),
        np.asarray(wk, np.float32), np.asarray(wv, np.float32),
        np.asarray(wo, np.float32), np.asarray(freqs_cos, np.float32),
        np.asarray(freqs_sin, np.float32), np.asarray(mask, np.float32))

    nc = _get_nc(causal)
    res = run_bass_kernel_spmd(nc, in_maps, core_ids=list(range(NCORES)))
    LAST_EXEC_NS = res.exec_time_ns

    acc = res.results[0]["out_part"].astype(np.float64)
    for c in range(1, NCORES):
        acc += res.results[c]["out_part"].astype(np.float64)
    return acc.astype(np.float32).reshape(B, S, DIM)


if __name__ == "__main__":
    rng = np.random.default_rng(0)
    inputs = {
        "x": rng.standard_normal((B, S, DIM), dtype=np.float32),
        "wq": (rng.standard_normal((DIM, DIM), dtype=np.float32) * 0.02),
        "wk": (rng.standard_normal((NKV * HD, DIM), dtype=np.float32) * 0.02),
        "wv": (rng.standard_normal((NKV * HD, DIM), dtype=np.float32) * 0.02),
        "wo": (rng.standard_normal((DIM, DIM), dtype=np.float32) * 0.02),
        "freqs_cos": rng.random((S, HD // 2), dtype=np.float32),
        "freqs_sin": rng.random((S, HD // 2), dtype=np.float32),
        "mask": np.triu(np.full((S, S), -1e9, dtype=np.float32), k=1),
        "start_pos": 0,
    }
    out = kernel(**inputs)
    print("out", out.shape, out.dtype, float(np.abs(out).mean()))


# revision 17
# speedup vs baseline: 1.2017x; 1.0336x over previous
"""Tensor-parallel llama-style attention (prefill) on 8 TRN2 NeuronCores.

bf16 version. All matmul operands are bf16: the PE runs bf16 at the same
1 cycle/row as f32r, but LDWEIGHTS halves (256 vs 512+ cycles per
[128,128] tile) so stationary loads hide completely under the 512-cycle
moving streams, and DMA/SBUF traffic halves. fp8 was evaluated and
rejected: e4m3's 3.6% per-element quantization error is multiplicative
into the output (sim: 5-9e-2 rel vs the 2e-2 budget; bf16 sims at 4e-3).

Sharding: tensor-parallel over heads. Core c holds q-heads [4c, 4c+4),
kv-head c, the matching rows of wq/wk/wv, and columns [512c, 512c+512) of
wo. Each core computes a full-size partial of the output projection;
partials are summed on the host (the "all-reduce after wo").

Device-side layout:
  - Activations keep the feature dim on partitions: xT [DIM, TOK],
    Q^T/K^T [128, S] per head, V in token-major chunks. Q is SBUF-
    resident in bf16 (no DRAM spill). wo is loaded once and stays
    resident.
  - RoPE: head-dim basis permuted on the host (even components first,
    odd second) -> half-partition swap + mul/add vs cos/sin tables,
    reading the projection result straight from PSUM.
  - Causal masking: the 4 distinct diagonal mask tiles (relative key
    offset 0/128/256/384 vs a 512-query block) are built once on-chip
    with gpsimd.affine_select and added on DVE - no mask DMA at all.
    Diagonal score tiles are also narrowed to the visible query range
    [128j, 512), cutting PE/ACT/DVE work on the diagonal by ~38% with
    no extra instructions; the skipped region is never read.
    Non-causal masks fall back to a DVE tensor_add of a DMA'd mask.
  - Softmax: no max-subtraction (scores*scale is O(10); exp in f32 is
    safe). Row sums via a ones-column matmul on the PE (DVE cannot
    reduce across partitions); 1/sum via reciprocal_approx_fast (5x
    faster than reciprocal, ~18 correct bits - plenty for 2e-2), then
    gpsimd.partition_broadcast.
"""

import math
import os
import sys

sys.path.insert(0, "/opt/trn_rl_repo")

import numpy as np
import ml_dtypes

import concourse.bacc as bacc
import concourse.tile as tile
import concourse.mybir as mybir
from concourse import masks
from concourse.bass_utils import run_bass_kernel_spmd

B, S, DIM = 2, 2048, 4096
TOK = B * S
NH, NKV, HD = 32, 8, 128
NCORES = 8
HQ = NH // NCORES            # 4 query heads per core
SCALE = 1.0 / math.sqrt(HD)
F32 = mybir.dt.float32
BF16 = mybir.dt.bfloat16
NP_BF = ml_dtypes.bfloat16
EXP = mybir.ActivationFunctionType.Exp
GE = mybir.AluOpType.is_ge

QB = 4          # q-blocks per batch (512 queries each)
QW = S // QB    # 512
KT = S // 128   # 16 k-tiles per batch
NJ = HQ + 2     # 6 projection output tiles: 4 Q heads, K, V


def _build(causal: bool):
    nc = bacc.Bacc("TRN2", target_bir_lowering=False, debug=False)

    xT_d = nc.dram_tensor("xT", [DIM, TOK], BF16, kind="ExternalInput")
    w_d = nc.dram_tensor("wqkvT", [DIM, NJ * HD], BF16, kind="ExternalInput")
    wo_d = nc.dram_tensor("woT", [HQ * HD, DIM], BF16, kind="ExternalInput")
    cos_d = nc.dram_tensor("cosT", [HD, S], F32, kind="ExternalInput")
    sin_d = nc.dram_tensor("sinTs", [HD, S], F32, kind="ExternalInput")
    if not causal:
        mask_d = nc.dram_tensor("maskTd", [QB * KT, 128, QW], F32,
                                kind="ExternalInput")
    out_d = nc.dram_tensor("out_part", [TOK, DIM], BF16,
                           kind="ExternalOutput")

    xT = xT_d.ap().rearrange("(kt p) t -> p kt t", p=128)      # [128, 32, TOK]
    w_ap = w_d.ap().rearrange("(kt p) j -> p kt j", p=128)     # [128, 32, 768]
    wo_ap = wo_d.ap().rearrange("(dt p) m -> p dt m", p=128)   # [128, 4, DIM]
    out_v = out_d.ap().rearrange("(g p) m -> p g m", p=128)    # [128, 32, DIM]

    with tile.TileContext(nc) as tc:
        with (
            tc.tile_pool(name="const", bufs=1) as const_pool,
            tc.tile_pool(name="batch", bufs=1) as batch_pool,
            tc.tile_pool(name="kv", bufs=2) as kv_pool,
        ):
            wqkv = const_pool.tile([128, 32, NJ * HD], BF16)
            for kc in range(4):     # chunked so P1 can start early
                nc.scalar.dma_start(wqkv[:, kc * 8:(kc + 1) * 8, :],
                                    w_ap[:, kc * 8:(kc + 1) * 8, :])
            wo_s = const_pool.tile([128, HQ, DIM], BF16)
            for mc in range(4):
                nc.scalar.dma_start(
                    wo_s[:, :, mc * 1024:(mc + 1) * 1024],
                    wo_ap[:, :, mc * 1024:(mc + 1) * 1024])
            ident = const_pool.tile([128, 128], F32)
            masks.make_identity(nc, ident[:])
            ones_f = const_pool.tile([128, 1], F32)
            nc.vector.memset(ones_f[:], 1.0)
            ones_col = const_pool.tile([128, 1], BF16)
            nc.vector.tensor_copy(ones_col[:], ones_f[:])
            if causal:
                # diagonal mask tiles built on-chip: tile j covers keys
                # [128j, 128j+128) vs queries [0, 512) relative to the
                # block diagonal; keep 0 where q >= p + 128j, else -1e9
                m4 = const_pool.tile([128, 4, QW], F32)
                nc.gpsimd.memset(m4[:], 0.0)
                for j in range(4):
                    nc.gpsimd.affine_select(
                        out=m4[:, j, :], in_=m4[:, j, :],
                        compare_op=GE, fill=-1e9,
                        base=-128 * j, pattern=[[1, QW]],
                        channel_multiplier=-1)

            # per-batch SBUF-resident activations; K/V double-buffered so
            # P1(b+1) can overwrite them while A(b) is still reading
            qh_s = batch_pool.tile([128, HQ, S], BF16)
            att_h = batch_pool.tile([128, HQ, S], BF16)

            with (
                tc.tile_pool(name="xt", bufs=2) as xt_pool,
                tc.tile_pool(name="cs", bufs=2) as cs_pool,
                tc.tile_pool(name="rope", bufs=2) as rope_pool,
                tc.tile_pool(name="vtmp", bufs=2) as vtmp_pool,
                tc.tile_pool(name="mask", bufs=2) as mask_pool,
                tc.tile_pool(name="pT", bufs=4) as p_pool,
                tc.tile_pool(name="rcp", bufs=2) as r_pool,
                tc.tile_pool(name="osb", bufs=2) as osb_pool,
            ):
              for b in range(B):
                kT_s = kv_pool.tile([128, S], BF16, tag="kT")
                v_s = kv_pool.tile([128, KT, HD], BF16, tag="v")
                # ---------- P1: QKV projections + RoPE ----------
                with (
                    tc.tile_pool(name="p1ps", bufs=NJ, space="PSUM") as p1ps,
                    tc.tile_pool(name="trps", bufs=2, space="PSUM") as trps,
                ):
                    for tb in range(4):          # 512-token chunks
                        c0 = b * S + tb * 512
                        sl = slice(tb * 512, tb * 512 + 512)
                        cos_c = cs_pool.tile([HD, 512], F32, tag="cos")
                        sin_c = cs_pool.tile([HD, 512], F32, tag="sin")
                        nc.sync.dma_start(cos_c[:], cos_d.ap()[:, sl])
                        nc.sync.dma_start(sin_c[:], sin_d.ap()[:, sl])
                        pss = [p1ps.tile([128, 512], F32, tag="ps",
                                         name=f"ps{j}")
                               for j in range(NJ)]
                        for ks in range(4):      # k slices of 8 x-tiles
                            xt_c = xt_pool.tile([128, 8, 512], BF16, tag="xt")
                            nc.sync.dma_start(
                                xt_c[:],
                                xT[:, ks * 8:(ks + 1) * 8, c0:c0 + 512])
                            for j in range(NJ):
                                for k in range(8):
                                    nc.tensor.matmul(
                                        pss[j][:],
                                        wqkv[:, ks * 8 + k,
                                             j * HD:(j + 1) * HD],
                                        xt_c[:, k, :],
                                        start=(ks == 0 and k == 0),
                                        stop=(ks == 3 and k == 7))
                        for j in range(NJ):
                            ps = pss[j]
                            if j < HQ + 1:
                                # RoPE: out = z*cos + swap64(z)*sin_signed
                                tmp = rope_pool.tile([128, 512], F32,
                                                     tag="tmp")
                                nc.vector.tensor_mul(
                                    tmp[0:64, :], ps[64:128, :],
                                    sin_c[0:64, :])
                                nc.vector.tensor_mul(
                                    tmp[64:128, :], ps[0:64, :],
                                    sin_c[64:128, :])
                                t2 = rope_pool.tile([128, 512], F32, tag="t2")
                                nc.vector.tensor_mul(t2[:], ps[:], cos_c[:])
                                if j < HQ:
                                    nc.vector.tensor_add(
                                        qh_s[:, j, sl], t2[:], tmp[:])
                                else:
                                    nc.vector.tensor_add(
                                        kT_s[:, sl], t2[:], tmp[:])
                            else:
                                # V: copy from PSUM, transpose to token-major
                                v_sb = vtmp_pool.tile([128, 512], F32)
                                nc.vector.tensor_copy(v_sb[:], ps[:])
                                for h2 in range(4):
                                    tp = trps.tile([128, 128], F32)
                                    nc.tensor.transpose(
                                        tp[:],
                                        v_sb[:, h2 * 128:(h2 + 1) * 128],
                                        ident[:])
                                    nc.vector.tensor_copy(
                                        v_s[:, tb * 4 + h2, :], tp[:])

                # ---------- A: attention (writes att_h in SBUF) ----------
                with (
                    tc.tile_pool(name="sps", bufs=4, space="PSUM") as sps,
                    tc.tile_pool(name="sums", bufs=2, space="PSUM") as sums_ps,
                    tc.tile_pool(name="ops", bufs=2, space="PSUM") as o_ps_pool,
                ):
                    for qb in range(QB):
                        if causal:
                            nkt = 4 * (qb + 1)
                            kt0 = 4 * qb
                        else:
                            m_s = mask_pool.tile([128, KT, QW], F32)
                            nc.scalar.dma_start(
                                m_s[:],
                                mask_d.ap()[qb * KT:(qb + 1) * KT]
                                .rearrange("kt p q -> p kt q"))
                            nkt = KT
                            kt0 = 0
                        for h in range(HQ):
                            sum_ps = sums_ps.tile([1, QW], F32)
                            o_ps = o_ps_pool.tile([128, QW], F32)
                            # software-pipelined by one kt: the PE issues
                            # scores(kt+1) while ACT exps pT(kt). Diagonal
                            # tiles are narrowed to the visible query range
                            # [128j, 512): the skipped region is never read.
                            prev = None
                            for kt in range(nkt):
                                w0 = (128 * (kt - kt0)
                                      if causal and kt >= kt0 else 0)
                                s_ps = sps.tile([128, QW], F32, tag="s_ps")
                                nc.tensor.matmul(
                                    s_ps[:, w0:],
                                    kT_s[:, kt * 128:(kt + 1) * 128],
                                    qh_s[:, h, qb * QW + w0:(qb + 1) * QW],
                                    start=True, stop=True)
                                if causal:
                                    if kt >= kt0:
                                        nc.vector.tensor_add(
                                            s_ps[:, w0:], s_ps[:, w0:],
                                            m4[:, kt - kt0, w0:])
                                else:
                                    nc.vector.tensor_add(
                                        s_ps[:], s_ps[:], m_s[:, kt, :])
                                pT = p_pool.tile([128, QW], BF16, tag="pT")
                                nc.scalar.activation(
                                    pT[:, w0:], s_ps[:, w0:], EXP, bias=0.0,
                                    scale=SCALE)
                                if prev is not None:
                                    pv, pkt, pw = prev
                                    nc.tensor.matmul(
                                        sum_ps[:, pw:], ones_col[:],
                                        pv[:, pw:],
                                        start=(pkt == 0), stop=False)
                                    nc.tensor.matmul(
                                        o_ps[:, pw:], v_s[:, pkt, :],
                                        pv[:, pw:],
                                        start=(pkt == 0), stop=False)
                                prev = (pT, kt, w0)
                            pv, pkt, pw = prev
                            nc.tensor.matmul(
                                sum_ps[:, pw:], ones_col[:], pv[:, pw:],
                                start=(pkt == 0), stop=True)
                            nc.tensor.matmul(
                                o_ps[:, pw:], v_s[:, pkt, :], pv[:, pw:],
                                start=(pkt == 0), stop=True)
                            rcp = r_pool.tile([1, QW], F32, tag="rcp")
                            nc.vector.reciprocal_approx_fast(
                                rcp[:], sum_ps[:])
                            bc_sb = r_pool.tile([128, QW], F32, tag="bc")
                            nc.gpsimd.partition_broadcast(bc_sb[:], rcp[:])
                            nc.vector.tensor_mul(
                                att_h[:, h, qb * QW:(qb + 1) * QW],
                                o_ps[:], bc_sb[:])

                # ---------- W: output projection partial ----------
                with (
                    tc.tile_pool(name="wps", bufs=5, space="PSUM") as wps,
                ):
                    for mb in range(8):          # 512-wide output columns
                        for tg in range(4):      # groups of 4 token tiles
                            o_sb = osb_pool.tile([128, 4, 512], BF16)
                            for ts in range(4):
                                tt = tg * 4 + ts
                                ps_w = wps.tile([128, 512], F32)
                                for d4 in range(HQ):
                                    nc.tensor.matmul(
                                        ps_w[:],
                                        att_h[:, d4, tt * 128:(tt + 1) * 128],
                                        wo_s[:, d4,
                                             mb * 512:(mb + 1) * 512],
                                        start=(d4 == 0), stop=(d4 == HQ - 1))
                                # split PSUM->SBUF casts between DVE and ACT
                                if ts % 2 == 0:
                                    nc.vector.tensor_copy(
                                        o_sb[:, ts, :], ps_w[:])
                                else:
                                    nc.scalar.activation(
                                        o_sb[:, ts, :], ps_w[:],
                                        mybir.ActivationFunctionType.Copy)
                            g0 = b * (S // 128) + tg * 4
                            nc.sync.dma_start(
                                out_v[:, g0:g0 + 4, mb * 512:(mb + 1) * 512],
                                o_sb[:])

    nc.compile()
    return nc


_CACHE = {}
LAST_EXEC_NS = None


def _get_nc(causal: bool):
    if causal not in _CACHE:
        _CACHE[causal] = _build(causal)
    return _CACHE[causal]


def _host_prep(x, wq, wk, wv, wo, freqs_cos, freqs_sin, mask):
    perm = np.concatenate([np.arange(0, HD, 2), np.arange(1, HD, 2)])
    wq_p = wq.reshape(NH, HD, DIM)[:, perm, :].reshape(NH * HD, DIM)
    wk_p = wk.reshape(NKV, HD, DIM)[:, perm, :].reshape(NKV * HD, DIM)

    xT = np.ascontiguousarray(x.reshape(TOK, DIM).T).astype(NP_BF)

    cos = freqs_cos.T                     # [64, S]
    sin = freqs_sin.T
    cosT = np.ascontiguousarray(np.concatenate([cos, cos], 0))       # [128, S]
    sinTs = np.ascontiguousarray(np.concatenate([-sin, sin], 0))

    ref_mask = np.triu(np.full((S, S), -1e9, dtype=np.float32), k=1)
    causal = np.array_equal(mask, ref_mask)

    if not causal:
        maskT = np.ascontiguousarray(mask.T) / np.float32(SCALE)   # [k, q]
        maskTd = np.empty((QB * KT, 128, QW), dtype=np.float32)
        for qb in range(QB):
            for j in range(KT):
                maskTd[qb * KT + j] = maskT[j * 128:(j + 1) * 128,
                                            qb * QW:(qb + 1) * QW]

    in_maps = []
    for c in range(NCORES):
        wqT = wq_p[c * HQ * HD:(c + 1) * HQ * HD, :].T          # [DIM, 512]
        wkT = wk_p[c * HD:(c + 1) * HD, :].T                    # [DIM, 128]
        wvT = wv[c * HD:(c + 1) * HD, :].T                      # [DIM, 128]
        wqkvT = np.ascontiguousarray(
            np.concatenate([wqT, wkT, wvT], 1)).astype(NP_BF)
        woT = np.ascontiguousarray(
            wo[:, c * HQ * HD:(c + 1) * HQ * HD].T).astype(NP_BF)
        m = {"xT": xT, "wqkvT": wqkvT, "woT": woT,
             "cosT": cosT, "sinTs": sinTs}
        if not causal:
            m["maskTd"] = maskTd
        in_maps.append(m)
    return causal, in_maps


def kernel(x, wq, wk, wv, wo, freqs_cos, freqs_sin, mask, start_pos):
    global LAST_EXEC_NS
    causal, in_maps = _host_prep(
        np.asarray(x, np.float32), np.asarray(wq, np.float32),
        np.asarray(wk, np.float32), np.asarray(wv, np.float32),
        np.asarray(wo, np.float32), np.asarray(freqs_cos, np.float32),
        np.asarray(freqs_sin, np.float32), np.asarray(mask, np.float32))

    nc = _get_nc(causal)
    res = run_bass_kernel_spmd(nc, in_maps, core_ids=list(range(NCORES)))
    LAST_EXEC_NS = res.exec_time_ns

    acc = res.results[0]["out_part"].astype(np.float64)
    for c in range(1, NCORES):
        acc += res.results[c]["out_part"].astype(np.float64)
    return acc.astype(np.float32).reshape(B, S, DIM)


if __name__ == "__main__":
    rng = np.random.default_rng(0)
    inputs = {
        "x": rng.standard_normal((B, S, DIM), dtype=np.float32),
        "wq": (rng.standard_normal((DIM, DIM), dtype=np.float32) * 0.02),
        "wk": (rng.standard_normal((NKV * HD, DIM), dtype=np.float32) * 0.02),
        "wv": (rng.standard_normal((NKV * HD, DIM), dtype=np.float32) * 0.02),
        "wo": (rng.standard_normal((DIM, DIM), dtype=np.float32) * 0.02),
        "freqs_cos": rng.random((S, HD // 2), dtype=np.float32),
        "freqs_sin": rng.random((S, HD // 2), dtype=np.float32),
        "mask": np.triu(np.full((S, S), -1e9, dtype=np.float32), k=1),
        "start_pos": 0,
    }
    out = kernel(**inputs)
    print("out", out.shape, out.dtype, float(np.abs(out).mean()))
